# revision 1
# baseline (speedup 1.0000x reference)
"""Trainium2 Bass kernel for nn_EpisodicMemory (trail_read_all, eval, 2 steps).

Sharding: data-parallel over BS — one batch-sample per NeuronCore (8 cores).
Per-bank params (tau/alpha/bias) are baked in as immediates at trace time.

Per-core computation (bs fixed), for each bank b and step t:
    scores = y @ K_b^T / tau_b            [N, M]
    U      = exp(scores)                  (unnormalized attn; Z = row-sum)
    dU     = U^T.T @ V_b                  [N, D]  (unnormalized delta, PSUM)
    dot    = sum(y * dU, -1) * rz / D     (rz = 1/Z)
    gate   = sigmoid(alpha_b * dot + bias_b)   (computed as 1/(1+exp(-x)))
    g      = gate * rz
    y'     = y + g * dU ;  out_acc += g * dU
Final output = sum over banks/steps of (g * dU) = sum_b (y_final_b - seed).

Matmul layouts: PE contracts over the partition dim, so the kernel keeps
transposed copies of y (yT: [d, n]) and K (kT: [d, m]) as matmul lhsT/rhs and
transposes U ([n, m] -> [m, n]) between the two matmuls.  Transposes go via
DMA-xbar (bf16 variant) or TensorE+identity (f32/f32r variants).
"""

import os

import numpy as np

import concourse.bass as bass
import concourse.mybir as mybir
import concourse.tile as tile
from concourse import bacc
from concourse.bass_utils import run_bass_kernel_spmd
from concourse.masks import make_identity

dt = mybir.dt
AL = mybir.AluOpType
AF = mybir.ActivationFunctionType

BS, B, M, D, N = 8, 4, 256, 256, 2048
P = 128
NT = N // P   # 16 row tiles of y
QB = 4        # n-tiles per gate batch (bounded by PSUM banks)
NQ = NT // QB
N_STEPS = 2

f32 = dt.float32


def _build(variant: str, tau, alpha, bias, use_mask: bool, reps: int = 1):
    DT = dt.bfloat16 if variant == "bf16" else f32
    # matmul-operand storage dtype; float32r = relaxed-precision PE mode
    # (1 cyc/row vs 4 for f32).  The BIR verifier requires producers of f32r
    # matmul operands to write rounded f32r, so the tiles are declared f32r.
    DTmm = dt.float32r if variant == "f32r" else DT
    xbar = variant == "bf16"

    def mm(ap):
        return ap

    nc = bacc.Bacc(None, target_bir_lowering=False)
    seed_d = nc.dram_tensor("seed", [N, D], f32, kind="ExternalInput")
    emk_d = nc.dram_tensor("em_K", [B, M, D], f32, kind="ExternalInput")
    emv_d = nc.dram_tensor("em_V", [B, M, D], f32, kind="ExternalInput")
    out_d = nc.dram_tensor("out", [N, D], f32, kind="ExternalOutput")
    if use_mask:
        msk_d = nc.dram_tensor("mask", [B, P, M], f32, kind="ExternalInput")

    with tile.TileContext(nc) as tc:
        import contextlib

        ctx = contextlib.ExitStack()
        with ctx:
            pool = lambda name, bufs, space="SBUF": ctx.enter_context(
                tc.tile_pool(name=name, bufs=bufs, space=space)
            )
            p_s = pool("p_s", NT)
            p_sdt = pool("p_sdt", NT) if xbar else None
            p_sT = pool("p_sT", NT)
            p_k = pool("p_k", B)
            p_v = pool("p_v", B)
            p_acc = pool("p_acc", NT)
            p_y1 = pool("p_y1", 2 * NT)
            p_y1T = pool("p_y1T", 2 * NT)
            p_U = pool("p_U", 6)
            p_uT = pool("p_uT", 6)
            p_stage = pool("p_stage", 4)
            p_scr = pool("p_scr", 4)
            p_tiny = pool("p_tiny", 32)
            p_ps = pool("p_ps", 8 if xbar else 6, space="PSUM")
            p_pt = None if xbar else pool("p_pt", 2, space="PSUM")
            p_const = pool("p_const", 1)
            p_msk = pool("p_msk", B) if use_mask else None

            ident = None
            if not xbar:
                ident = p_const.tile([P, P], f32, name="ident")
                make_identity(nc, ident)

            def transp_to(dst, srcs):
                """dst[:, c:c+128] = transpose(src) for (src, c) in srcs."""
                if xbar:
                    for src, c in srcs:
                        nc.sync.dma_start(dst[:, c : c + P], src, transpose=True)
                else:
                    w = max(c for _, c in srcs) + P
                    pt = p_pt.tile([P, 512], f32, name="pt")
                    for src, c in srcs:
                        nc.tensor.transpose(pt[:, c : c + P], src, ident)
                    nc.vector.tensor_copy(dst[:, 0:w], pt[:, 0:w])

            for rep in range(reps):
                # ---- preload ----
                sb_s = []
                s_src = []  # transpose source for seed (needs DT dtype)
                for i in range(NT):
                    s_i = p_s.tile([P, D], f32, name="s_i")
                    nc.gpsimd.dma_start(s_i, seed_d[i * P : (i + 1) * P, :])
                    sb_s.append(s_i)
                    if xbar:
                        sdt_i = p_sdt.tile([P, D], DT, name="sdt_i")
                        nc.gpsimd.dma_start(sdt_i, seed_d[i * P : (i + 1) * P, :])
                        s_src.append(sdt_i)
                    else:
                        s_src.append(s_i)

                msk = []
                if use_mask:
                    for b in range(B):
                        m_b = p_msk.tile([P, M], f32, name="m_b")
                        nc.gpsimd.dma_start(m_b, msk_d[b])
                        msk.append(m_b)

                v = []
                kT = []
                for b in range(B):
                    v_b = p_v.tile([P, 2 * D], DTmm, name="v_b")
                    for mh in range(2):
                        if DTmm == dt.float32r:
                            ev_t = p_stage.tile([P, D], f32, name="ev_t")
                            nc.gpsimd.dma_start(
                                ev_t, emv_d[b, mh * P : (mh + 1) * P, :]
                            )
                            nc.vector.tensor_copy(v_b[:, mh * D : (mh + 1) * D], ev_t)
                        else:
                            nc.gpsimd.dma_start(
                                v_b[:, mh * D : (mh + 1) * D],
                                emv_d[b, mh * P : (mh + 1) * P, :],
                            )
                    v.append(v_b)
                    ek = []
                    for mt in range(2):
                        ek_t = p_stage.tile([P, D], DT, name="ek_t")
                        nc.gpsimd.dma_start(ek_t, emk_d[b, mt * P : (mt + 1) * P, :])
                        ek.append(ek_t)
                    kT_b = p_k.tile([P, 2 * M], DTmm, name="kT_b")
                    transp_to(
                        kT_b,
                        [
                            (ek[0][:, 0:P], 0),
                            (ek[0][:, P : 2 * P], 2 * P),
                            (ek[1][:, 0:P], P),
                            (ek[1][:, P : 2 * P], 3 * P),
                        ],
                    )
                    kT.append(kT_b)

                sT = []
                for i in range(NT):
                    sT_i = p_sT.tile([P, 2 * P], DTmm, name="sT_i")
                    transp_to(sT_i, [(s_src[i][:, 0:P], 0), (s_src[i][:, P : 2 * P], P)])
                    sT.append(sT_i)

                acc = [None] * NT

                # ---- main loop ----
                y1_cur, y1T_cur = None, None
                for b in range(B):
                    for t in range(N_STEPS):
                        lhsT = sT if t == 0 else y1T_cur
                        yprev = sb_s if t == 0 else y1_cur
                        y1_new, y1T_new = [], []
                        for q in range(NQ):
                            zs = p_tiny.tile([P, QB], f32, name="zs")
                            dots = p_tiny.tile([P, QB], f32, name="dots")
                            pss = []
                            for j in range(QB):
                                i = q * QB + j
                                ps = p_ps.tile([P, 512], f32, name="ps")
                                pss.append(ps)
                                nc.tensor.matmul(
                                    ps[:, 0:M], mm(lhsT[i][:, 0:P]), mm(kT[b][:, 0:M]),
                                    start=True, stop=False,
                                )
                                nc.tensor.matmul(
                                    ps[:, 0:M], mm(lhsT[i][:, P : 2 * P]), mm(kT[b][:, M : 2 * M]),
                                    start=False, stop=True,
                                )
                                U = p_U.tile([P, M], DT, name="U")
                                if use_mask:
                                    nc.scalar.activation(U, ps[:, 0:M], AF.Exp, scale=1.0 / tau[b])
                                    nc.vector.tensor_tensor(U, U, msk[b], AL.mult)
                                    nc.vector.tensor_reduce(
                                        zs[:, j : j + 1], U, mybir.AxisListType.X, AL.add
                                    )
                                else:
                                    nc.scalar.activation(
                                        U, ps[:, 0:M], AF.Exp,
                                        scale=1.0 / tau[b], accum_out=zs[:, j : j + 1],
                                    )
                                uT = p_uT.tile([P, 2 * P], DTmm, name="uT")
                                transp_to(uT, [(U[:, 0:P], 0), (U[:, P : 2 * P], P)])
                                nc.tensor.matmul(
                                    ps[:, M : M + D], mm(uT[:, 0:P]), mm(v[b][:, 0:D]),
                                    start=True, stop=False,
                                )
                                nc.tensor.matmul(
                                    ps[:, M : M + D], mm(uT[:, P : 2 * P]), mm(v[b][:, D : 2 * D]),
                                    start=False, stop=True,
                                )
                                scr = p_scr.tile([P, D], f32, name="scr")
                                nc.vector.scalar_tensor_tensor(
                                    scr, ps[:, M : M + D], 1.0, yprev[i],
                                    AL.bypass, AL.mult, accum_out=dots[:, j : j + 1],
                                )
                            rzs = p_tiny.tile([P, QB], f32, name="rzs")
                            nc.vector.reciprocal(rzs, zs)
                            dn = p_tiny.tile([P, QB], f32, name="dn")
                            nc.vector.tensor_tensor(dn, dots, rzs, AL.mult)
                            e1 = p_tiny.tile([P, QB], f32, name="e1")
                            nc.scalar.activation(
                                e1, dn, AF.Exp, scale=-alpha[b] / D, bias=-bias[b]
                            )
                            ge = p_tiny.tile([P, QB], f32, name="ge")
                            nc.vector.tensor_scalar_add(ge, e1, 1.0)
                            gate = p_tiny.tile([P, QB], f32, name="gate")
                            nc.vector.reciprocal(gate, ge)
                            g = p_tiny.tile([P, QB], f32, name="g")
                            nc.vector.tensor_tensor(g, gate, rzs, AL.mult)
                            for j in range(QB):
                                i = q * QB + j
                                ps = pss[j]
                                gj = g[:, j : j + 1]
                                if b == 0 and t == 0:
                                    a_i = p_acc.tile([P, D], f32, name="a_i")
                                    nc.vector.tensor_scalar(
                                        a_i, ps[:, M : M + D], gj, None, AL.mult
                                    )
                                    acc[i] = a_i
                                else:
                                    nc.vector.scalar_tensor_tensor(
                                        acc[i], ps[:, M : M + D], gj, acc[i], AL.mult, AL.add
                                    )
                                if t == 0:
                                    y1_i = p_y1.tile([P, D], DT, name="y1_i")
                                    nc.vector.scalar_tensor_tensor(
                                        y1_i, ps[:, M : M + D], gj, yprev[i], AL.mult, AL.add
                                    )
                                    y1T_i = p_y1T.tile([P, 2 * P], DTmm, name="y1T_i")
                                    transp_to(
                                        y1T_i, [(y1_i[:, 0:P], 0), (y1_i[:, P : 2 * P], P)]
                                    )
                                    y1_new.append(y1_i)
                                    y1T_new.append(y1T_i)
                        if t == 0:
                            y1_cur, y1T_cur = y1_new, y1T_new

                for i in range(NT):
                    nc.gpsimd.dma_start(out_d[i * P : (i + 1) * P, :], acc[i])

    nc.compile()
    return nc


def kernel(**inputs):
    seed = np.ascontiguousarray(np.asarray(inputs["seed"], dtype=np.float32))
    em_K = np.ascontiguousarray(np.asarray(inputs["em_K"], dtype=np.float32))
    em_V = np.ascontiguousarray(np.asarray(inputs["em_V"], dtype=np.float32))
    em_S = np.asarray(inputs["em_S"], dtype=np.float32)
    gate_alpha = np.asarray(inputs["gate_alpha"], dtype=np.float32)
    gate_bias = np.asarray(inputs["gate_bias"], dtype=np.float32)
    raw_tau = np.asarray(inputs["raw_tau"], dtype=np.float32)

    variant = os.environ.get("EM_VARIANT", "f32r")
    tau = [float(np.log1p(np.exp(raw_tau[b])) + 0.1) for b in range(B)]
    alpha = [float(gate_alpha[b]) for b in range(B)]
    bias = [float(gate_bias[b]) for b in range(B)]
    use_mask = bool((em_S <= 0).any())

    nc = _build(variant, tau, alpha, bias, use_mask)

    in_maps = []
    for c in range(BS):
        m = {"seed": seed[c], "em_K": em_K[c], "em_V": em_V[c]}
        if use_mask:
            mask = (em_S[c] > 0).astype(np.float32)  # [B, M]
            m["mask"] = np.ascontiguousarray(
                np.broadcast_to(mask[:, None, :], (B, P, M))
            )
        in_maps.append(m)

    res = run_bass_kernel_spmd(nc, in_maps, core_ids=list(range(BS)))
    out = np.stack([res.results[c]["out"] for c in range(BS)], axis=0)
    return out.astype(np.float32)

